# revision 1
# baseline (speedup 1.0000x reference)
"""Trainium2 Bass kernel for BandProcessorWithHistory.

Reference computation (per full inputs):
    xn = LN(x, g1, be1); Q = xn@Wq.T + bq
    K = history@Wk.T + bk; V = history@Wv.T + bv          # [T,H,D], shared over batch
    scores = einsum('btd,thd->bth', Q, K)/sqrt(D) + log(decay + 1e-10)
    attn = softmax(scores, -1); attended = einsum('bth,thd->btd', attn, V)
    x2 = x + attended@Wo.T + bo
    out = x2 + gelu(LN(x2,g2,be2)@W1.T + b1)@W2.T + b2

Sharding: the T (sequence) axis is split over 8 NeuronCores (256 positions
each). Attention is per-position over its own history column, so the split is
embarrassingly parallel (history rows and x columns partition cleanly; weights
replicated). Each core runs an identical program on its shard.

Per-core layout: activations are kept feature-major ("transposed", [D, rows])
so every matmul contracts over the SBUF partition dim. Rows are t-major
(r = t_local*B + b). K/V for a block of 16 positions are produced on-chip and
consumed immediately (never spilled to DRAM). The softmax decay bias and the
block-diagonal structure are folded into one constant multiplicative mask:
softmax(s + log(d+1e-10)) == normalize(exp(s) * (d+1e-10), masked).
"""

import math
import os
from contextlib import ExitStack

import numpy as np

import concourse.bacc as bacc
import concourse.bass as bass
import concourse.mybir as mybir
import concourse.tile as tile
from concourse.bass_utils import run_bass_kernel_spmd
from concourse.masks import make_identity

F32 = mybir.dt.float32
F32R = mybir.dt.float32r

B, T, H, D = 8, 2048, 64, 512
N_CORES = 8
T_LOC = T // N_CORES          # 256 positions per core
R = B * T_LOC                 # 2048 activation rows per core (r = t*B + b)
HR = T_LOC * H                # 16384 history rows per core
P = 128
DC = D // P                   # 4 chunks of the model dim
D2 = 2 * D                    # FFN hidden
D2C = D2 // P                 # 8 chunks
BLK_T = 16                    # positions per attention block
N_BLK = T_LOC // BLK_T        # 16 blocks
BCOL = BLK_T * B              # 128 activation cols per block
HCOL = BLK_T * H              # 1024 history cols per block
RB = 512                      # r-columns per projection block
N_RB = R // RB                # 4
DECAY_RATE = 0.95
LN_EPS = 1e-5

# float32r streams the PE at 1 cycle/row (vs 4 for plain fp32) when the
# moving dim is >=256. Numerics are validated against the jax reference on HW.
USE_F32R = os.environ.get("KERNEL_MM_DTYPE", "f32r") == "f32r"

MDT = F32R if USE_F32R else F32  # dtype of every tensor feeding a matmul

_last_result = [None]  # BassKernelResults of the most recent kernel() call
_cached = {}           # compiled program cache


def _mm(nc, out, lhsT, rhs, start, stop):
    nc.tensor.matmul(out, lhsT, rhs, start=start, stop=stop)


def _ln_stats(nc, pools, xtile, stats_ps, tag):
    """Column LN statistics of one [128, DC, RB] chunk (feature-major).

    Returns (rs, sh): rsqrt(var+eps) row and mu*rs row, both [1, RB] MDT.
    """
    sb = pools["sb"]
    ones_col = pools["ones_col"]
    lb = pools.get("ln_bufs", 1)

    if xtile.dtype == MDT:
        xr = xtile
    else:
        # stats matmuls need an operand rounded to the matmul dtype
        xr = sb.tile([P, DC, RB], MDT, tag=f"xr{tag}",
                     bufs=pools.get("sq_bufs", 1))
        nc.gpsimd.tensor_copy(xr[:], xtile[:])

    # ones_col holds 1/D, so these accumulate the column means directly:
    # ps_sum = mean(x), ps_sq = mean(x^2)
    ps_sum = stats_ps.tile([1, RB], F32, tag="ps_st", bufs=4)
    ps_sq = stats_ps.tile([1, RB], F32, tag="ps_st", bufs=4)
    for dc in range(DC):
        _mm(nc, ps_sum[:], ones_col[:], xr[:, dc], dc == 0, dc == DC - 1)
    sq = sb.tile([P, DC, RB], MDT, tag=f"sq{tag}", bufs=pools.get("sq_bufs", 1))
    nc.scalar.square(sq[:], xtile[:])
    for dc in range(DC):
        _mm(nc, ps_sq[:], ones_col[:], sq[:, dc], dc == 0, dc == DC - 1)

    # per-column stats on partition 0, packed along the free dim
    st = sb.tile([1, 4, RB], F32, tag=f"st{tag}", bufs=pools.get("st_bufs", lb))
    mu, var, std, rsf = st[:, 0], st[:, 1], st[:, 2], st[:, 3]
    nc.vector.tensor_copy(mu, ps_sum[:])
    nc.vector.tensor_tensor(std, ps_sum[:], mu, mybir.AluOpType.mult)  # mu^2
    # var + eps = (ex2 + eps) - mu^2
    nc.vector.scalar_tensor_tensor(var, ps_sq[:], pools["eps1"][:], std,
                                   mybir.AluOpType.add,
                                   mybir.AluOpType.subtract)
    nc.scalar.activation(std, var, mybir.ActivationFunctionType.Sqrt)
    nc.vector.reciprocal_approx_fast(rsf, std)
    # rs feeds matmuls -> rounded copy at SBUF base partition 0
    rs = sb.tile([1, RB], MDT, tag=f"rs{tag}", bufs=lb)
    with nc.allow_low_precision(reason="fp32r matmul operand"):
        nc.vector.tensor_copy(rs[:], rsf)

    # mu*rs row for the rank-1 shift matmul
    sh = sb.tile([1, RB], MDT, tag=f"sh{tag}", bufs=lb)
    with nc.allow_low_precision(reason="fp32r matmul operand"):
        nc.vector.tensor_tensor(sh[:], mu, rsf, mybir.AluOpType.mult)
    return rs, sh


def _ln_apply(nc, pools, xtile, gb_row, rs, sh, bcast_ps, out_tile):
    """out[:, dc] = x*(g (x) rs) + be - (g (x) mu*rs), all feature-major."""
    be_col = pools["be_col"]
    for dc in range(DC):
        ps_a = bcast_ps.tile([P, RB], F32, tag="bc", bufs=2)
        ps_b = bcast_ps.tile([P, RB], F32, tag="bc", bufs=2)
        # scale[p, r] = g[dc*128+p] * rs[r]
        _mm(nc, ps_a[:], gb_row[:, dc * P : (dc + 1) * P], rs[:], True, True)
        # shift[p, r] = g[d] * mu[r] * rs[r]
        _mm(nc, ps_b[:], gb_row[:, dc * P : (dc + 1) * P], sh[:], True, True)
        nc.vector.tensor_tensor(out_tile[:, dc], xtile[:, dc], ps_a[:],
                                mybir.AluOpType.mult)
        # out = (x*scale + be) - shift
        nc.vector.scalar_tensor_tensor(
            out_tile[:, dc], out_tile[:, dc], be_col[:, dc : dc + 1], ps_b[:],
            mybir.AluOpType.add, mybir.AluOpType.subtract)


def _build_program():
    nc = bacc.Bacc("TRN2", target_bir_lowering=False, debug=False)

    xT = nc.dram_tensor("xT", [P, DC, R], MDT, kind="ExternalInput")
    xTf = nc.dram_tensor("xTf", [P, DC, R], F32, kind="ExternalInput")
    histT = nc.dram_tensor("histT", [P, DC, HR], MDT, kind="ExternalInput")
    wq = nc.dram_tensor("wq", [P, DC, D], MDT, kind="ExternalInput")
    wk = nc.dram_tensor("wk", [P, DC, D], MDT, kind="ExternalInput")
    wv = nc.dram_tensor("wv", [P, DC, D], MDT, kind="ExternalInput")
    wo = nc.dram_tensor("wo", [P, DC, D], MDT, kind="ExternalInput")
    w1 = nc.dram_tensor("w1", [P, DC, D2], MDT, kind="ExternalInput")
    w2 = nc.dram_tensor("w2", [P, D2C, D], MDT, kind="ExternalInput")
    bqd = nc.dram_tensor("bq", [P, DC], F32, kind="ExternalInput")
    bkd = nc.dram_tensor("bk", [P, DC], F32, kind="ExternalInput")
    bod = nc.dram_tensor("bo", [P, DC], F32, kind="ExternalInput")
    b1d = nc.dram_tensor("b1", [P, D2C], F32, kind="ExternalInput")
    b2d = nc.dram_tensor("b2", [P, DC], F32, kind="ExternalInput")
    gb1d = nc.dram_tensor("gb1", [1, D], MDT, kind="ExternalInput")
    gb2d = nc.dram_tensor("gb2", [1, D], MDT, kind="ExternalInput")
    be1d = nc.dram_tensor("be1v", [P, DC], F32, kind="ExternalInput")
    be2d = nc.dram_tensor("be2v", [P, DC], F32, kind="ExternalInput")
    onesd = nc.dram_tensor("ones", [P, 1], MDT, kind="ExternalInput")
    maskd = nc.dram_tensor("mask", [P, HCOL], F32, kind="ExternalInput")
    outT = nc.dram_tensor("outT", [P, DC, R], F32, kind="ExternalOutput")

    with tile.TileContext(nc) as tc, ExitStack() as top:
        const = top.enter_context(tc.tile_pool(name="const", bufs=1))
        pers = top.enter_context(tc.tile_pool(name="pers", bufs=1))

        # --- constants resident for the whole kernel ---
        wk_t = const.tile([P, DC, D], MDT)
        wv_t = const.tile([P, DC, D], MDT)
        bk_t = const.tile([P, DC], F32)
        ident = const.tile([P, P], F32)
        ones_col = const.tile([P, 1], MDT)
        eps1 = const.tile([1, 1], F32)
        gb1_t = const.tile([1, D], MDT)
        gb2_t = const.tile([1, D], MDT)
        be1_t = const.tile([P, DC], F32)
        be2_t = const.tile([P, DC], F32)
        nc.sync.dma_start(ones_col[:], onesd[:])
        nc.sync.dma_start(gb1_t[:], gb1d[:])
        nc.sync.dma_start(be1_t[:], be1d[:])
        make_identity(nc, ident[:])
        nc.vector.memset(eps1[:], LN_EPS)

        def load_bulk_consts():
            nc.sync.dma_start(wk_t[:], wk[:])
            nc.sync.dma_start(wv_t[:], wv[:])
            nc.sync.dma_start(bk_t[:], bkd[:])
            nc.sync.dma_start(gb2_t[:], gb2d[:])
            nc.sync.dma_start(be2_t[:], be2d[:])
            nc.sync.dma_start(wo_t[:], wo[:])
            nc.sync.dma_start(bo_t[:], bod[:])

        pools = {"const": const, "ones_col": ones_col, "eps1": eps1}

        ATT = pers.tile([P, DC, R], MDT)
        wo_t = pers.tile([P, DC, D], MDT)
        bo_t = pers.tile([P, DC], F32)

        mid = ExitStack()
        qt_pool = mid.enter_context(tc.tile_pool(name="qt", bufs=1))
        QT = qt_pool.tile([P, DC, R], MDT)

        # ---------------- Stage A: LN1 + Q projection ----------------
        with ExitStack() as ctx:
            apool = ctx.enter_context(tc.tile_pool(name="stage_a", bufs=2))
            stats_ps = ctx.enter_context(
                tc.tile_pool(name="a_stats", bufs=1, space="PSUM"))
            bcast_ps = ctx.enter_context(
                tc.tile_pool(name="a_bcast", bufs=1, space="PSUM"))
            mm_ps = ctx.enter_context(
                tc.tile_pool(name="a_mm", bufs=2, space="PSUM"))

            apools = dict(pools)
            apools["sb"] = apool
            apools["ln_bufs"] = 3
            apools["st_bufs"] = 3
            apools["sq_bufs"] = 2
            wq_t = apool.tile([P, DC, D], MDT, tag="wq", bufs=1)
            bq_t = apool.tile([P, DC], F32, tag="bq", bufs=1)

            apools["be_col"] = be1_t
            astate = {}

            def a_front(rb):
                xt = apool.tile([P, DC, RB], MDT, tag="xt", bufs=3)
                nc.sync.dma_start(xt[:], xT[:, :, rb * RB : (rb + 1) * RB])
                astate[rb] = (xt,) + _ln_stats(nc, apools, xt, stats_ps, "a")

            def a_back(rb):
                xt, rs, sh = astate.pop(rb)
                xn = apool.tile([P, DC, RB], MDT, tag="xn", bufs=2)
                _ln_apply(nc, apools, xt, gb1_t, rs, sh, bcast_ps, xn)
                for oc in range(DC):
                    ps = mm_ps.tile([P, RB], F32, tag="mm")
                    for dc in range(DC):
                        _mm(nc, ps[:], wq_t[:, dc, oc * P : (oc + 1) * P],
                            xn[:, dc], dc == 0, dc == DC - 1)
                    nc.scalar.activation(
                        QT[:, oc, rb * RB : (rb + 1) * RB], ps[:],
                        mybir.ActivationFunctionType.Identity,
                        bias=bq_t[:, oc : oc + 1])

            a_front(0)
            nc.sync.dma_start(wq_t[:], wq[:])
            nc.sync.dma_start(bq_t[:], bqd[:])
            a_front(1)
            load_bulk_consts()
            for rb in range(N_RB):
                if rb + 2 < N_RB:
                    a_front(rb + 2)
                a_back(rb)

        # ---------------- Stage B/C: fused K/V projection + attention ----------------
        with ExitStack() as ctx:
            hpool = ctx.enter_context(tc.tile_pool(name="attn_sb", bufs=1))
            mask_t = hpool.tile([P, HCOL], F32, tag="mask", bufs=1)
            nc.sync.dma_start(mask_t[:], maskd[:])
            mm_ps = ctx.enter_context(
                tc.tile_pool(name="kv_mm", bufs=3, space="PSUM"))
            sc_ps = ctx.enter_context(
                tc.tile_pool(name="scores", bufs=1, space="PSUM"))
            tr_ps = ctx.enter_context(
                tc.tile_pool(name="trans", bufs=2, space="PSUM"))
            at_ps = ctx.enter_context(
                tc.tile_pool(name="attend", bufs=1, space="PSUM"))

            for blk in range(N_BLK):
                c0 = blk * HCOL            # history col offset
                r0 = blk * BCOL            # activation col offset

                ht = hpool.tile([P, DC, HCOL], MDT, tag="hist", bufs=2)
                nc.sync.dma_start(ht[:], histT[:, :, c0 : c0 + HCOL])

                # K^T block [d=512 over (P,oc), (t,h)=1024]
                kt = hpool.tile([P, DC, HCOL], MDT, tag="kv", bufs=3)
                for oc in range(DC):
                    for nb in range(2):
                        ps = mm_ps.tile([P, RB], F32, tag="mm")
                        for dc in range(DC):
                            _mm(nc, ps[:], wk_t[:, dc, oc * P : (oc + 1) * P],
                                ht[:, dc, nb * RB : (nb + 1) * RB],
                                dc == 0, dc == DC - 1)
                        nc.scalar.activation(
                            kt[:, oc, nb * RB : (nb + 1) * RB], ps[:],
                            mybir.ActivationFunctionType.Identity,
                            bias=bk_t[:, oc : oc + 1])

                # V block, row-major [(t,h)=1024 over (P,rc), d=512]
                vt = hpool.tile([P, D2C, D], MDT, tag="kv", bufs=3)
                for rc in range(D2C):
                    ps = mm_ps.tile([P, D], F32, tag="mm")
                    for dc in range(DC):
                        _mm(nc, ps[:], ht[:, dc, rc * P : (rc + 1) * P],
                            wv_t[:, dc], dc == 0, dc == DC - 1)
                    nc.vector.tensor_copy(vt[:, rc], ps[:])

                # scores [r=128, (t,h)=1024]
                sc = sc_ps.tile([P, HCOL], F32, tag="sc")
                for nb in range(2):
                    for dc in range(DC):
                        _mm(nc, sc[:, nb * RB : (nb + 1) * RB],
                            QT[:, dc, r0 : r0 + BCOL],
                            kt[:, dc, nb * RB : (nb + 1) * RB],
                            dc == 0, dc == DC - 1)
                ex = hpool.tile([P, HCOL], F32, tag="ex", bufs=2)
                nc.scalar.activation(ex[:], sc[:],
                                     mybir.ActivationFunctionType.Exp)

                # mask+decay multiply, then row-sum + reciprocal
                am = hpool.tile([P, HCOL], F32, tag="am", bufs=2)
                dn = hpool.tile([P, 2], F32, tag="dn", bufs=2)
                nc.vector.tensor_tensor(am[:], ex[:], mask_t[:],
                                        mybir.AluOpType.mult)
                nc.vector.tensor_reduce(dn[:, 0:1], am[:],
                                        mybir.AxisListType.X,
                                        mybir.AluOpType.add)
                nc.vector.reciprocal_approx_fast(dn[:, 1:2], dn[:, 0:1])

                # transpose attn -> [(t,h), r]
                at = hpool.tile([P, D2C, P], MDT, tag="at", bufs=2)
                for half in range(2):
                    pst = tr_ps.tile([P, 4, P], F32, tag="pst")
                    for q in range(4):
                        cc = half * 4 + q
                        nc.tensor.transpose(pst[:, q], am[:, cc * P : (cc + 1) * P],
                                            ident[:])
                    nc.vector.tensor_copy(at[:, half * 4 : half * 4 + 4], pst[:])

                # attended [r=128, d=512], normalized on eviction
                ps_at = at_ps.tile([P, D], F32, tag="atps")
                for cc in range(D2C):
                    _mm(nc, ps_at[:], at[:, cc], vt[:, cc], cc == 0, cc == D2C - 1)
                ats = hpool.tile([P, D], F32, tag="ats", bufs=2)
                nc.vector.tensor_scalar_mul(ats[:], ps_at[:], dn[:, 1:2])

                # transpose attended into feature-major resident ATT
                pst = tr_ps.tile([P, 4, P], F32, tag="pst")
                for qc in range(DC):
                    nc.tensor.transpose(pst[:, qc], ats[:, qc * P : (qc + 1) * P],
                                        ident[:])
                nc.scalar.copy(ATT[:, :, r0 : r0 + BCOL], pst[:])

        mid.close()

        # ---------------- Stage D/E: O-projection + residual + LN2 + FFN ----------------
        with ExitStack() as ctx:
            epool = ctx.enter_context(tc.tile_pool(name="stage_e", bufs=1))
            stats_ps = ctx.enter_context(
                tc.tile_pool(name="e_stats", bufs=1, space="PSUM"))
            bcast_ps = ctx.enter_context(
                tc.tile_pool(name="e_bcast", bufs=1, space="PSUM"))
            mm_ps = ctx.enter_context(
                tc.tile_pool(name="e_mm", bufs=2, space="PSUM"))

            w1_t = epool.tile([P, DC, D2], MDT, tag="w1", bufs=1)
            w2_t = epool.tile([P, D2C, D], MDT, tag="w2", bufs=1)
            b1_t = epool.tile([P, D2C], F32, tag="b1", bufs=1)
            b2_t = epool.tile([P, DC], F32, tag="b2", bufs=1)
            nc.sync.dma_start(w1_t[:], w1[:])
            nc.sync.dma_start(w2_t[:], w2[:])
            nc.sync.dma_start(b1_t[:], b1d[:])
            nc.sync.dma_start(b2_t[:], b2d[:])

            epools = dict(pools)
            epools["sb"] = epool
            epools["ln_bufs"] = 2
            epools["st_bufs"] = 2

            estate = {}

            def e_front(rb):
                rs_ = slice(rb * RB, (rb + 1) * RB)
                xt = epool.tile([P, DC, RB], F32, tag="xt2", bufs=2)
                nc.sync.dma_start(xt[:], xTf[:, :, rs_])

                # x2 = x + attended @ Wo.T + bo'
                x2 = epool.tile([P, DC, RB], F32, tag="x2", bufs=3)
                for oc in range(DC):
                    ps = mm_ps.tile([P, RB], F32, tag="mm")
                    for dc in range(DC):
                        _mm(nc, ps[:], wo_t[:, dc, oc * P : (oc + 1) * P],
                            ATT[:, dc, rs_], dc == 0, dc == DC - 1)
                    nc.vector.scalar_tensor_tensor(
                        x2[:, oc], ps[:], bo_t[:, oc : oc + 1], xt[:, oc],
                        mybir.AluOpType.add, mybir.AluOpType.add)
                estate[rb] = (x2,) + _ln_stats(nc, epools, x2, stats_ps, "e")

            def e_back(rb):
                rs_ = slice(rb * RB, (rb + 1) * RB)
                x2, rs, sh = estate.pop(rb)
                h2 = epool.tile([P, DC, RB], MDT, tag="h2", bufs=1)
                _ln_apply(nc, epools, x2, gb2_t, rs, sh, bcast_ps, h2)

                # a1 = gelu(h2 @ W1.T + b1)  (feature-major [1024, RB])
                a1 = epool.tile([P, D2C, RB], MDT, tag="a1", bufs=1)
                for oc in range(D2C):
                    ps = mm_ps.tile([P, RB], F32, tag="mm")
                    for dc in range(DC):
                        _mm(nc, ps[:], w1_t[:, dc, oc * P : (oc + 1) * P],
                            h2[:, dc], dc == 0, dc == DC - 1)
                    nc.scalar.activation(a1[:, oc], ps[:],
                                         mybir.ActivationFunctionType.Gelu,
                                         bias=b1_t[:, oc : oc + 1])

                # out = x2 + a1 @ W2.T + b2
                ot = epool.tile([P, DC, RB], F32, tag="ot", bufs=1)
                for oc in range(DC):
                    ps = mm_ps.tile([P, RB], F32, tag="mm")
                    for dc in range(D2C):
                        _mm(nc, ps[:], w2_t[:, dc, oc * P : (oc + 1) * P],
                            a1[:, dc], dc == 0, dc == D2C - 1)
                    nc.vector.scalar_tensor_tensor(
                        ot[:, oc], ps[:], b2_t[:, oc : oc + 1], x2[:, oc],
                        mybir.AluOpType.add, mybir.AluOpType.add)
                nc.sync.dma_start(outT[:, :, rs_], ot[:])

            epools["be_col"] = be2_t
            e_front(0)
            e_front(1)
            for rb in range(N_RB):
                if rb + 2 < N_RB:
                    e_front(rb + 2)
                e_back(rb)

    nc.compile()
    return nc


def _tile_fm(a):
    """[Dred, N] feature-major -> [128, Dred//128, N] device tiling."""
    dred, n = a.shape
    return np.ascontiguousarray(
        a.reshape(dred // P, P, n).swapaxes(0, 1)).astype(np.float32)


def _vec_pc(v):
    """[n*128] bias vector -> [128, n] (partition, chunk)."""
    return np.ascontiguousarray(v.reshape(-1, P).T).astype(np.float32)


def _make_weight_map(inputs):
    inv_sqrt_d = 1.0 / math.sqrt(D)
    wq = _tile_fm(inputs["Wq"].T * inv_sqrt_d)
    wk = _tile_fm(inputs["Wk"].T)
    wv = _tile_fm(inputs["Wv"].T)
    wo = _tile_fm(inputs["Wo"].T)
    w1 = _tile_fm(inputs["W1"].T)
    w2 = _tile_fm(inputs["W2"].T)
    bq = _vec_pc(inputs["bq"] * inv_sqrt_d)
    bk = _vec_pc(inputs["bk"])
    # bv folded into bo: softmax rows sum to 1, so attn @ (V + 1 (x) bv) @ Wo.T
    # contributes exactly bv @ Wo.T per row.
    bo = _vec_pc(inputs["bo"] + inputs["Wo"].astype(np.float64) @
                 inputs["bv"].astype(np.float64))
    b1 = _vec_pc(inputs["b1"])
    b2 = _vec_pc(inputs["b2"])
    gb1 = np.ascontiguousarray(inputs["g1"][None, :]).astype(np.float32)
    gb2 = np.ascontiguousarray(inputs["g2"][None, :]).astype(np.float32)
    be1v = _vec_pc(inputs["be1"])
    be2v = _vec_pc(inputs["be2"])
    ones = np.full((P, 1), 1.0 / D, dtype=np.float32)

    h = np.arange(H)
    decay = (DECAY_RATE ** (H - 1 - h).astype(np.float64)) + 1e-10
    mask = np.zeros((P, HCOL), dtype=np.float32)
    for p_ in range(P):
        i = p_ // B
        mask[p_, i * H : (i + 1) * H] = decay
    return dict(wq=wq, wk=wk, wv=wv, wo=wo, w1=w1, w2=w2, bq=bq, bk=bk,
                bo=bo, b1=b1, b2=b2, gb1=gb1, gb2=gb2, be1v=be1v, be2v=be2v,
                ones=ones, mask=mask)


def kernel(**inputs):
    inputs = {k: np.asarray(v, dtype=np.float32) for k, v in inputs.items()}
    if "nc" not in _cached:
        _cached["nc"] = _build_program()
    nc = _cached["nc"]

    wmap = _make_weight_map(inputs)
    x, history = inputs["x"], inputs["history"]

    in_maps = []
    for c in range(N_CORES):
        ts = slice(c * T_LOC, (c + 1) * T_LOC)
        # t-major rows: r = t_local*B + b
        xr = x[:, ts, :].transpose(1, 0, 2).reshape(R, D)
        hr = history[ts].reshape(HR, D)
        m = dict(wmap)
        m["xT"] = _tile_fm(np.ascontiguousarray(xr.T))
        m["xTf"] = m["xT"]
        m["histT"] = _tile_fm(np.ascontiguousarray(hr.T))
        in_maps.append(m)

    res = run_bass_kernel_spmd(nc, in_maps, core_ids=list(range(N_CORES)))
    _last_result[0] = res

    out = np.empty((B, T, D), dtype=np.float32)
    for c in range(N_CORES):
        ts = slice(c * T_LOC, (c + 1) * T_LOC)
        ot = res.results[c]["outT"]                      # [128, DC, R]
        full = ot.swapaxes(0, 1).reshape(D, R).T         # [R, D], r = t*B+b
        out[:, ts, :] = full.reshape(T_LOC, B, D).transpose(1, 0, 2)
    return out



# revision 6
# speedup vs baseline: 1.0387x; 1.0387x over previous
"""Trainium2 Bass kernel for BandProcessorWithHistory (v2, algebraic rewrite).

Reference computation (full inputs):
    xn = LN(x, g1, be1); Q = xn@Wq.T + bq
    K = history@Wk.T + bk; V = history@Wv.T + bv          # [T,H,D]
    scores = einsum('btd,thd->bth', Q, K)/sqrt(D) + log(decay + 1e-10)
    attn = softmax(scores, -1); attended = einsum('bth,thd->btd', attn, V)
    x2 = x + attended@Wo.T + bo
    out = x2 + gelu(LN(x2,g2,be2)@W1.T + b1)@W2.T + b2

Key algebraic rewrite (removes the dominant K/V projections, 8x less work):
    scores  = Q.(H Wk^T)^T = (xn @ (Wq^T Wk)/sqrt(D)) . H^T        (+ bq@Wk;
              bk drops: softmax is invariant to a per-row constant)
    attended@Wo^T = (attn @ H) @ (Wo Wv)^T + Wo@bv   (rows of attn sum to 1)
so only [2048 x 512 x 512]-shaped projections remain per core, contracting
with raw history directly.  The per-position decay bias + block-diagonal
validity mask are folded into one additive f32 constant added to the scores
PSUM before exp; exp's accum_out produces softmax denominators for free.

Sharding: T (sequence) axis split over 8 cores (256 positions each);
embarrassingly parallel.  Activations feature-major [d on partitions, rows
on free dim], rows r = t_local*B + b.

Precision: matmuls run fp8e4(e4m3) with DoubleRow perf mode (2 k-chunks of
128 packed per instruction, ~1.5-2x PE throughput); weight matrices are
pre-scaled on the host into the fp8 normal range and descaled via activation
/vector-op scale constants.  LN statistics come from fp8 ones-matmuls;
row-wise LN scale/shift broadcasts use k=1 f32r rank-1 matmuls (exact).
Residual x rides in bf16 (with bo' pre-added on host); output is bf16.
Attention probabilities transpose via the DMA xbar (bf16) instead of the PE.
"""

import math
import os
from contextlib import ExitStack

import numpy as np
import ml_dtypes

import concourse.bacc as bacc
import concourse.bass as bass
import concourse.mybir as mybir
import concourse.tile as tile
from concourse.bass_utils import run_bass_kernel_spmd

F32 = mybir.dt.float32
F32R = mybir.dt.float32r
BF16 = mybir.dt.bfloat16
FP8 = mybir.dt.float8e4
DR = mybir.MatmulPerfMode.DoubleRow

B, T, H, D = 8, 2048, 64, 512
N_CORES = 8
T_LOC = T // N_CORES          # 256 positions per core
R = B * T_LOC                 # 2048 activation rows per core (r = t*B + b)
HR = T_LOC * H                # 16384 history rows per core
P = 128
DC = D // P                   # 4 chunks of the model dim
D2 = 2 * D
D2C = D2 // P                 # 8 chunks
BLK_T = 16                    # positions per attention block
N_BLK = T_LOC // BLK_T        # 16 blocks
HCOL = BLK_T * H              # 1024 history cols per block
RB = 512                      # r-columns per projection block
N_RB = R // RB                # 4
DECAY_RATE = 0.95
LN_EPS = 1e-5

# fp8 weight pre-scales (descaled on-chip via activation/vector constants)
S_A = 4096.0                  # LN1-folded Wq^T.Wk weight
S_Q = 256.0                   # Q2 activation scale (descaled inside exp)
S_O = 512.0                   # Wo.Wv weight
S_1 = 128.0                   # W1
S_2 = 128.0                   # W2
NEG_BIG = -30.0               # additive mask for invalid score entries

_last_result = [None]
_cached = {}

AF = mybir.ActivationFunctionType
OP = mybir.AluOpType


def _build_program():
    nc = bacc.Bacc("TRN2", target_bir_lowering=False, debug=False)

    xq8d = nc.dram_tensor("xq8", [P, DC, R], FP8, kind="ExternalInput")
    xb16d = nc.dram_tensor("xb16", [P, DC, R], BF16, kind="ExternalInput")
    hfmd = nc.dram_tensor("hfm8", [P, DC, HR], FP8, kind="ExternalInput")
    hrmd = nc.dram_tensor("hrm8", [P, HR // P, D], FP8, kind="ExternalInput")
    wad = nc.dram_tensor("wa8", [P, DC, D], FP8, kind="ExternalInput")
    wovd = nc.dram_tensor("wov8", [P, DC, D], FP8, kind="ExternalInput")
    w1d = nc.dram_tensor("w18", [P, DC, D2], FP8, kind="ExternalInput")
    w2d = nc.dram_tensor("w28", [P, D2C, D], FP8, kind="ExternalInput")
    maskd = nc.dram_tensor("maskS", [P, HCOL], F32, kind="ExternalInput")
    dqd = nc.dram_tensor("dq", [P, DC], F32, kind="ExternalInput")
    b1cd = nc.dram_tensor("b1c", [P, D2C], F32, kind="ExternalInput")
    b2cd = nc.dram_tensor("b2c", [P, DC], F32, kind="ExternalInput")
    be2d = nc.dram_tensor("be2c", [P, DC], F32, kind="ExternalInput")
    onesAd = nc.dram_tensor("onesA", [1, P], F32R, kind="ExternalInput")
    cqnd = nc.dram_tensor("cqn", [1, D], F32R, kind="ExternalInput")
    g2rd = nc.dram_tensor("g2r", [1, D], F32R, kind="ExternalInput")
    ones8d = nc.dram_tensor("ones8", [P, 2, 16], FP8, kind="ExternalInput")
    outd = nc.dram_tensor("outT", [P, DC, R], BF16, kind="ExternalOutput")

    with tile.TileContext(nc) as tc, ExitStack() as top:
        const = top.enter_context(tc.tile_pool(name="const", bufs=1))
        pers = top.enter_context(tc.tile_pool(name="pers", bufs=1))

        # ---- constants resident for the whole kernel ----
        wa_t = const.tile([P, DC, D], FP8)
        ones8_t = const.tile([P, 2, 16], FP8)
        eps1 = const.tile([1, 1], F32)
        nc.vector.memset(eps1[:], LN_EPS)
        onesA_t = const.tile([1, P], F32R)
        cqn_t = const.tile([1, D], F32R)
        dq_t = const.tile([P, DC], F32)
        mask_t = const.tile([P, HCOL], F32)
        wov_t = const.tile([P, DC, D], FP8)
        w1_t = const.tile([P, DC, D2], FP8)
        w2_t = const.tile([P, D2C, D], FP8)
        g2r_t = const.tile([1, D], F32R)
        b1c_t = const.tile([P, D2C], F32)
        b2c_t = const.tile([P, DC], F32)
        be2_t = const.tile([P, DC], F32)

        nc.sync.dma_start(wa_t[:], wad[:])
        nc.sync.dma_start(ones8_t[:], ones8d[:])
        nc.sync.dma_start(onesA_t[:], onesAd[:])
        nc.sync.dma_start(cqn_t[:], cqnd[:])
        nc.sync.dma_start(dq_t[:], dqd[:])

        def load_late_consts():
            nc.sync.dma_start(mask_t[:], maskd[:])
            nc.sync.dma_start(wov_t[:], wovd[:])
            nc.sync.dma_start(w1_t[:], w1d[:])
            nc.sync.dma_start(w2_t[:], w2d[:])
            nc.sync.dma_start(g2r_t[:], g2rd[:])
            nc.sync.dma_start(b1c_t[:], b1cd[:])
            nc.sync.dma_start(b2c_t[:], b2cd[:])
            nc.sync.dma_start(be2_t[:], be2d[:])

        # ---- per-core resident activations ----
        xq8_t = pers.tile([P, DC, R], FP8)
        xb16_t = pers.tile([P, DC, R], BF16)
        q2_t = pers.tile([P, DC, R], FP8)      # scaled by S_Q
        attb_t = pers.tile([P, DC, R], BF16)   # attended, feature-major
        att8_t = pers.tile([P, DC, R], FP8)

        nc.sync.dma_start(xq8_t[:], xq8d[:])
        nc.sync.dma_start(xb16_t[:], xb16d[:])

        def ln_stats(pool, stats_ps, src8, sq8, tag, bufs=2):
            """LN stats of one [128, DC, RB] fp8 chunk -> (rs, sh) f32r rows.

            src8: fp8 source; sq8: same-shape scratch for squares.
            """
            nc.gpsimd.tensor_tensor(sq8[:], src8, src8, OP.mult)
            ps = stats_ps.tile([16, 2, RB], F32, tag="st", bufs=bufs)
            for pr in range(2):
                nc.tensor.matmul(ps[:, 0], ones8_t[:],
                                 src8[:, 2 * pr : 2 * pr + 2],
                                 start=pr == 0, stop=pr == 1, perf_mode=DR)
            for pr in range(2):
                nc.tensor.matmul(ps[:, 1], ones8_t[:],
                                 sq8[:, 2 * pr : 2 * pr + 2],
                                 start=pr == 0, stop=pr == 1, perf_mode=DR)
            st = pool.tile([1, 3, RB], F32, tag=f"st{tag}", bufs=3)
            mu, m2, var = st[:, 0], st[:, 1], st[:, 2]
            nc.vector.tensor_scalar(mu, ps[0:1, 0], 1.0 / D, None, OP.mult)
            nc.gpsimd.tensor_tensor(m2, mu, mu, OP.mult)
            nc.vector.scalar_tensor_tensor(var, ps[0:1, 1], 1.0 / D, m2,
                                           OP.mult, OP.subtract)
            std = st[:, 1]  # reuse m2 slot
            nc.scalar.activation(std, var, AF.Sqrt, bias=eps1[:])
            rsf = st[:, 2]  # reuse var slot
            nc.vector.reciprocal_approx_fast(rsf, std)
            rssh = pool.tile([1, 2, RB], F32R, tag=f"rs{tag}", bufs=3)
            with nc.allow_low_precision(reason="f32r matmul operand"):
                nc.gpsimd.tensor_copy(rssh[:, 0], rsf)
                nc.gpsimd.tensor_tensor(rssh[:, 1], mu, rsf, OP.mult)
            return rssh

        # ================= Stage A: LN1 + Q2 projection =================
        with ExitStack() as ctx:
            apool = ctx.enter_context(tc.tile_pool(name="stage_a", bufs=2))
            stats_ps = ctx.enter_context(
                tc.tile_pool(name="a_stats", bufs=2, space="PSUM"))
            bc_ps = ctx.enter_context(
                tc.tile_pool(name="a_bcast", bufs=2, space="PSUM"))
            mm_ps = ctx.enter_context(
                tc.tile_pool(name="a_mm", bufs=2, space="PSUM"))

            astate = {}

            def a_front(rb):
                rsl = slice(rb * RB, (rb + 1) * RB)
                sq8 = apool.tile([P, DC, RB], FP8, tag="sq", bufs=2)
                astate[rb] = ln_stats(apool, stats_ps, xq8_t[:, :, rsl],
                                      sq8, "a")

            def a_back(rb):
                rsl = slice(rb * RB, (rb + 1) * RB)
                rssh = astate.pop(rb)
                ps_rsb = bc_ps.tile([P, RB], F32, tag="bc", bufs=2)
                nc.tensor.matmul(ps_rsb[:], onesA_t[:], rssh[:, 0],
                                 start=True, stop=True)
                rsb = apool.tile([P, RB], F32, tag="rsb", bufs=2)
                nc.scalar.copy(rsb[:], ps_rsb[:])
                for oc in range(DC):
                    ps_y = mm_ps.tile([P, RB], F32, tag="mm", bufs=2)
                    for pr in range(2):
                        nc.tensor.matmul(
                            ps_y[:], wa_t[:, 2 * pr : 2 * pr + 2,
                                          oc * P : (oc + 1) * P],
                            xq8_t[:, 2 * pr : 2 * pr + 2, rsl],
                            start=pr == 0, stop=pr == 1, perf_mode=DR)
                    ps_csh = bc_ps.tile([P, RB], F32, tag="bc", bufs=2)
                    nc.tensor.matmul(ps_csh[:],
                                     cqn_t[:, oc * P : (oc + 1) * P],
                                     rssh[:, 1], start=True, stop=True)
                    t = apool.tile([P, RB], F32, tag="t", bufs=2)
                    nc.vector.tensor_tensor(t[:], ps_y[:], rsb[:], OP.mult)
                    with nc.allow_low_precision(reason="fp8 activation"):
                        nc.vector.scalar_tensor_tensor(
                            q2_t[:, oc, rsl], t[:], dq_t[:, oc : oc + 1],
                            ps_csh[:], OP.add, OP.add)

            a_front(0)
            a_front(1)
            load_late_consts()
            for rb in range(N_RB):
                if rb + 2 < N_RB:
                    a_front(rb + 2)
                a_back(rb)

        # ================= Stage B/C: attention =================
        with ExitStack() as ctx:
            hpool = ctx.enter_context(tc.tile_pool(name="attn_sb", bufs=1))
            sc_ps = ctx.enter_context(
                tc.tile_pool(name="scores", bufs=2, space="PSUM"))
            ah_ps = ctx.enter_context(
                tc.tile_pool(name="attend", bufs=2, space="PSUM"))

            for blk in range(N_BLK):
                c0 = blk * HCOL
                r0 = blk * P

                hf = hpool.tile([P, DC, HCOL], FP8, tag="hf", bufs=3)
                nc.sync.dma_start(hf[:], hfmd[:, :, c0 : c0 + HCOL])
                hrt = hpool.tile([P, D2C, D], FP8, tag="hr", bufs=3)
                nc.sync.dma_start(hrt[:], hrmd[:, blk * D2C : (blk + 1) * D2C, :])

                ps_sc = sc_ps.tile([P, HCOL], F32, tag="sc", bufs=2)
                for nb in range(2):
                    for pr in range(2):
                        nc.tensor.matmul(
                            ps_sc[:, nb * RB : (nb + 1) * RB],
                            q2_t[:, 2 * pr : 2 * pr + 2, r0 : r0 + P],
                            hf[:, 2 * pr : 2 * pr + 2, nb * RB : (nb + 1) * RB],
                            start=pr == 0, stop=pr == 1, perf_mode=DR)
                nc.vector.tensor_tensor(ps_sc[:], ps_sc[:], mask_t[:], OP.add)

                am = hpool.tile([P, HCOL], BF16, tag="am", bufs=2)
                den = hpool.tile([P, 2], F32, tag="den", bufs=2)
                with nc.allow_low_precision(reason="bf16 attn probs"):
                    nc.scalar.activation(am[:], ps_sc[:], AF.Exp,
                                         scale=1.0 / S_Q,
                                         accum_out=den[:, 0:1])
                nc.vector.reciprocal_approx_fast(den[:, 1:2], den[:, 0:1])

                amT = hpool.tile([P, D2C, P], BF16, tag="amT", bufs=2)
                for ch in range(D2C):
                    nc.sync.dma_start(amT[:, ch], am[:, ch * P : (ch + 1) * P],
                                      transpose=True)
                amT8 = hpool.tile([P, D2C, P], FP8, tag="amT8", bufs=2)
                with nc.allow_low_precision(reason="fp8 attn probs"):
                    nc.vector.tensor_copy(amT8[:], amT[:])

                ps_ah = ah_ps.tile([P, D], F32, tag="ah", bufs=2)
                for pr in range(4):
                    nc.tensor.matmul(ps_ah[:], amT8[:, 2 * pr : 2 * pr + 2],
                                     hrt[:, 2 * pr : 2 * pr + 2],
                                     start=pr == 0, stop=pr == 3, perf_mode=DR)
                atb = hpool.tile([P, D], BF16, tag="atb", bufs=2)
                with nc.allow_low_precision(reason="bf16 attended"):
                    nc.vector.tensor_scalar_mul(atb[:], ps_ah[:], den[:, 1:2])
                for ec in range(DC):
                    nc.sync.dma_start(attb_t[:, ec, r0 : r0 + P],
                                      atb[:, ec * P : (ec + 1) * P],
                                      transpose=True)
                with nc.allow_low_precision(reason="fp8 attended"):
                    nc.vector.tensor_copy(att8_t[:, :, r0 : r0 + P],
                                          attb_t[:, :, r0 : r0 + P])

        # ================= Stage D/E: Wov + LN2 + FFN =================
        with ExitStack() as ctx:
            epool = ctx.enter_context(tc.tile_pool(name="stage_e", bufs=2))
            stats_ps = ctx.enter_context(
                tc.tile_pool(name="e_stats", bufs=1, space="PSUM"))
            bc_ps = ctx.enter_context(
                tc.tile_pool(name="e_bcast", bufs=1, space="PSUM"))
            mm_ps = ctx.enter_context(
                tc.tile_pool(name="e_mm", bufs=3, space="PSUM"))

            estate = {}

            def e_front(rb):
                rsl = slice(rb * RB, (rb + 1) * RB)
                x2 = epool.tile([P, DC, RB], F32, tag="x2", bufs=3)
                for oc in range(DC):
                    ps_w = mm_ps.tile([P, RB], F32, tag="mm", bufs=3)
                    for pr in range(2):
                        nc.tensor.matmul(
                            ps_w[:], wov_t[:, 2 * pr : 2 * pr + 2,
                                           oc * P : (oc + 1) * P],
                            att8_t[:, 2 * pr : 2 * pr + 2, rsl],
                            start=pr == 0, stop=pr == 1, perf_mode=DR)
                    nc.vector.scalar_tensor_tensor(
                        x2[:, oc], ps_w[:], 1.0 / S_O, xb16_t[:, oc, rsl],
                        OP.mult, OP.add)
                x28 = epool.tile([P, DC, RB], FP8, tag="x28", bufs=2)
                with nc.allow_low_precision(reason="fp8 stats input"):
                    nc.gpsimd.tensor_copy(x28[:], x2[:])
                sq8 = epool.tile([P, DC, RB], FP8, tag="sq", bufs=2)
                estate[rb] = (x2,) + (ln_stats(epool, stats_ps, x28[:],
                                               sq8, "e", bufs=1),)

            def e_back(rb):
                rsl = slice(rb * RB, (rb + 1) * RB)
                x2, rssh = estate.pop(rb)
                h28 = epool.tile([P, DC, RB], FP8, tag="h2", bufs=2)
                for dc in range(DC):
                    ps_ab = bc_ps.tile([P, 2, RB], F32, tag="bc", bufs=1)
                    nc.tensor.matmul(ps_ab[:, 0],
                                     g2r_t[:, dc * P : (dc + 1) * P],
                                     rssh[:, 0], start=True, stop=True)
                    nc.tensor.matmul(ps_ab[:, 1],
                                     g2r_t[:, dc * P : (dc + 1) * P],
                                     rssh[:, 1], start=True, stop=True)
                    with nc.allow_low_precision(reason="fp8 LN2 out"):
                        nc.vector.tensor_tensor(h28[:, dc], x2[:, dc],
                                                ps_ab[:, 0], OP.mult)
                        nc.vector.scalar_tensor_tensor(
                            h28[:, dc], h28[:, dc], be2_t[:, dc : dc + 1],
                            ps_ab[:, 1], OP.add, OP.subtract)

                a18 = epool.tile([P, D2C, RB], FP8, tag="a1", bufs=2)
                for oc in range(D2C):
                    ps_f = mm_ps.tile([P, RB], F32, tag="mm", bufs=3)
                    for pr in range(2):
                        nc.tensor.matmul(
                            ps_f[:], w1_t[:, 2 * pr : 2 * pr + 2,
                                          oc * P : (oc + 1) * P],
                            h28[:, 2 * pr : 2 * pr + 2],
                            start=pr == 0, stop=pr == 1, perf_mode=DR)
                    with nc.allow_low_precision(reason="fp8 gelu"):
                        nc.scalar.activation(a18[:, oc], ps_f[:], AF.Gelu,
                                             bias=b1c_t[:, oc : oc + 1],
                                             scale=1.0 / S_1)

                ot = epool.tile([P, DC, RB], BF16, tag="ot", bufs=2)
                tf = epool.tile([P, RB], F32, tag="tf", bufs=3)
                for oc in range(DC):
                    ps_f = mm_ps.tile([P, RB], F32, tag="mm", bufs=3)
                    for pr in range(4):
                        nc.tensor.matmul(
                            ps_f[:], w2_t[:, 2 * pr : 2 * pr + 2,
                                          oc * P : (oc + 1) * P],
                            a18[:, 2 * pr : 2 * pr + 2],
                            start=pr == 0, stop=pr == 3, perf_mode=DR)
                    nc.scalar.activation(tf[:], ps_f[:], AF.Identity,
                                         bias=b2c_t[:, oc : oc + 1],
                                         scale=1.0 / S_2)
                    with nc.allow_low_precision(reason="bf16 output"):
                        nc.vector.tensor_tensor(ot[:, oc], tf[:], x2[:, oc],
                                                OP.add)
                nc.sync.dma_start(outd[:, :, rsl], ot[:])

            e_front(0)
            e_front(1)
            for rb in range(N_RB):
                if rb + 2 < N_RB:
                    e_front(rb + 2)
                e_back(rb)

    nc.compile()
    return nc


def _tile_fm(a, dt):
    """[Dred, N] feature-major -> [128, Dred//128, N] device tiling."""
    dred, n = a.shape
    return np.ascontiguousarray(
        a.reshape(dred // P, P, n).swapaxes(0, 1)).astype(dt)


def _vec_pc(v):
    """[n*128] vector -> [128, n] (partition, chunk) f32."""
    return np.ascontiguousarray(np.asarray(v, np.float64).reshape(-1, P).T
                                ).astype(np.float32)


def _make_weight_map(inputs):
    f64 = {k: np.asarray(v, np.float64) for k, v in inputs.items()}
    isd = 1.0 / math.sqrt(D)

    WQK = (f64["Wq"].T @ f64["Wk"]) * isd          # [f, e]
    WA = f64["g1"][:, None] * WQK
    cq = WA.sum(axis=0)                            # [e]
    dq = f64["be1"] @ WQK + f64["bq"] @ f64["Wk"] * isd
    WOV = f64["Wo"] @ f64["Wv"]                    # [d, f]
    bo_p = f64["bo"] + f64["Wo"] @ f64["bv"]

    wa8 = _tile_fm(WA * S_A, ml_dtypes.float8_e4m3)
    wov8 = _tile_fm(WOV.T * S_O, ml_dtypes.float8_e4m3)
    w18 = _tile_fm(f64["W1"].T * S_1, ml_dtypes.float8_e4m3)
    w28 = _tile_fm(f64["W2"].T * S_2, ml_dtypes.float8_e4m3)

    h = np.arange(H)
    logdecay = np.log(DECAY_RATE ** (H - 1 - h) + 1e-10)
    maskS = np.full((P, HCOL), NEG_BIG * S_Q, np.float64)
    for p_ in range(P):
        t = p_ // B
        maskS[p_, t * H : (t + 1) * H] = S_Q * logdecay
    ones8 = np.ones((P, 2, 16), ml_dtypes.float8_e4m3)

    return dict(
        wa8=wa8, wov8=wov8, w18=w18, w28=w28,
        maskS=maskS.astype(np.float32),
        dq=_vec_pc(dq * S_Q),
        b1c=_vec_pc(f64["b1"]),
        b2c=_vec_pc(f64["b2"]),
        be2c=_vec_pc(f64["be2"]),
        onesA=np.full((1, P), S_Q / S_A, np.float32),
        cqn=(-cq * S_Q)[None, :].astype(np.float32),
        g2r=np.ascontiguousarray(f64["g2"][None, :]).astype(np.float32),
        ones8=ones8,
        _bo_p=bo_p,  # consumed by core_input_map, not a dram tensor
    )


def core_input_map(inputs, wmap, c):
    """Per-core input dict (core c owns positions [c*T_LOC, (c+1)*T_LOC))."""
    x = np.asarray(inputs["x"], np.float32)
    history = np.asarray(inputs["history"], np.float32)
    ts = slice(c * T_LOC, (c + 1) * T_LOC)
    xr = x[:, ts, :].transpose(1, 0, 2).reshape(R, D)      # r = t*B + b
    hr = history[ts].reshape(HR, D)
    m = {k: v for k, v in wmap.items() if not k.startswith("_")}
    m["xq8"] = _tile_fm(np.ascontiguousarray(xr.T), ml_dtypes.float8_e4m3)
    m["xb16"] = _tile_fm(np.ascontiguousarray((xr + wmap["_bo_p"]).T),
                         ml_dtypes.bfloat16)
    m["hfm8"] = _tile_fm(np.ascontiguousarray(hr.T), ml_dtypes.float8_e4m3)
    m["hrm8"] = np.ascontiguousarray(
        hr.reshape(HR // P, P, D).swapaxes(0, 1)).astype(ml_dtypes.float8_e4m3)
    return m


def unpack_out(ot):
    """[128, DC, R] bf16 device tile -> [B, T_LOC, D] f32."""
    full = np.asarray(ot, np.float32).swapaxes(0, 1).reshape(D, R).T
    return full.reshape(T_LOC, B, D).transpose(1, 0, 2)


def kernel(**inputs):
    if "nc" not in _cached:
        _cached["nc"] = _build_program()
    nc = _cached["nc"]

    wmap = _make_weight_map(inputs)
    in_maps = [core_input_map(inputs, wmap, c) for c in range(N_CORES)]

    res = run_bass_kernel_spmd(nc, in_maps, core_ids=list(range(N_CORES)))
    _last_result[0] = res

    out = np.empty((B, T, D), dtype=np.float32)
    for c in range(N_CORES):
        ts = slice(c * T_LOC, (c + 1) * T_LOC)
        out[:, ts, :] = unpack_out(res.results[c]["outT"])
    return out


# revision 11
# speedup vs baseline: 1.6969x; 1.6337x over previous
"""Trainium2 Bass kernel for BandProcessorWithHistory (v2, algebraic rewrite).

Reference computation (full inputs):
    xn = LN(x, g1, be1); Q = xn@Wq.T + bq
    K = history@Wk.T + bk; V = history@Wv.T + bv          # [T,H,D]
    scores = einsum('btd,thd->bth', Q, K)/sqrt(D) + log(decay + 1e-10)
    attn = softmax(scores, -1); attended = einsum('bth,thd->btd', attn, V)
    x2 = x + attended@Wo.T + bo
    out = x2 + gelu(LN(x2,g2,be2)@W1.T + b1)@W2.T + b2

Key algebraic rewrite (removes the dominant K/V projections, 8x less work):
    scores  = Q.(H Wk^T)^T = (xn @ (Wq^T Wk)/sqrt(D)) . H^T        (+ bq@Wk;
              bk drops: softmax is invariant to a per-row constant)
    attended@Wo^T = (attn @ H) @ (Wo Wv)^T + Wo@bv   (rows of attn sum to 1)
so only [2048 x 512 x 512]-shaped projections remain per core, contracting
with raw history directly.  The per-position decay bias + block-diagonal
validity mask are folded into one additive f32 constant added to the scores
PSUM before exp; exp's accum_out produces softmax denominators for free.

Sharding: T (sequence) axis split over 8 cores (256 positions each);
embarrassingly parallel.  Activations feature-major [d on partitions, rows
on free dim], rows r = t_local*B + b.

Precision: matmuls run fp8e4(e4m3) with DoubleRow perf mode (2 k-chunks of
128 packed per instruction, ~1.5-2x PE throughput); weight matrices are
pre-scaled on the host into the fp8 normal range and descaled via activation
/vector-op scale constants.  LN statistics come from fp8 ones-matmuls;
row-wise LN scale/shift broadcasts use k=1 f32r rank-1 matmuls (exact).
Residual x rides in bf16 (with bo' pre-added on host); output is bf16.
Attention probabilities transpose via the DMA xbar (bf16) instead of the PE.
"""

import math
import os
from contextlib import ExitStack

import numpy as np
import ml_dtypes

import concourse.bacc as bacc
import concourse.bass as bass
import concourse.mybir as mybir
import concourse.tile as tile
from concourse.bass_utils import run_bass_kernel_spmd

F32 = mybir.dt.float32
F32R = mybir.dt.float32r
BF16 = mybir.dt.bfloat16
FP8 = mybir.dt.float8e4
DR = mybir.MatmulPerfMode.DoubleRow

B, T, H, D = 8, 2048, 64, 512
N_CORES = 8
T_LOC = T // N_CORES          # 256 positions per core
R = B * T_LOC                 # 2048 activation rows per core (r = t*B + b)
HR = T_LOC * H                # 16384 history rows per core
P = 128
DC = D // P                   # 4 chunks of the model dim
D2 = 2 * D
D2C = D2 // P                 # 8 chunks
BLK_T = 16                    # positions per attention block
N_BLK = T_LOC // BLK_T        # 16 blocks
HCOL = BLK_T * H              # 1024 history cols per block
RB = 512                      # r-columns per projection block
N_RB = R // RB                # 4
DECAY_RATE = 0.95
LN_EPS = 1e-5

# fp8 weight pre-scales (descaled on-chip via activation/vector constants)
S_A = 4096.0                  # LN1-folded Wq^T.Wk weight
S_Q = 256.0                   # Q2 activation scale (descaled inside exp)
S_O = 512.0                   # Wo.Wv weight
S_1 = 128.0                   # W1
S_2 = 128.0                   # W2
NEG_BIG = -30.0               # additive mask for invalid score entries

_last_result = [None]
_cached = {}

AF = mybir.ActivationFunctionType
OP = mybir.AluOpType


def _build_program():
    nc = bacc.Bacc("TRN2", target_bir_lowering=False, debug=False)

    xq8d = nc.dram_tensor("xq8", [P, DC, R], FP8, kind="ExternalInput")
    xb16d = nc.dram_tensor("xb16", [P, DC, R], BF16, kind="ExternalInput")
    hfmd = nc.dram_tensor("hfm8", [P, DC, HR], FP8, kind="ExternalInput")
    hrmd = nc.dram_tensor("hrm8", [P, HR // P, D], FP8, kind="ExternalInput")
    wad = nc.dram_tensor("wa8", [P, DC, D], FP8, kind="ExternalInput")
    wovd = nc.dram_tensor("wov8", [P, DC, D], FP8, kind="ExternalInput")
    w1d = nc.dram_tensor("w18", [P, DC, D2], FP8, kind="ExternalInput")
    w2d = nc.dram_tensor("w28", [P, D2C, D], FP8, kind="ExternalInput")
    maskd = nc.dram_tensor("maskS", [P, HCOL], F32, kind="ExternalInput")
    dqd = nc.dram_tensor("dq", [P, DC], F32, kind="ExternalInput")
    b1cd = nc.dram_tensor("b1c", [P, D2C], F32, kind="ExternalInput")
    b2cd = nc.dram_tensor("b2c", [P, DC], F32, kind="ExternalInput")
    be2d = nc.dram_tensor("be2c", [P, DC], F32, kind="ExternalInput")
    onesAd = nc.dram_tensor("onesA", [1, P], F32R, kind="ExternalInput")
    cqnd = nc.dram_tensor("cqn", [1, D], F32R, kind="ExternalInput")
    g2rd = nc.dram_tensor("g2r", [1, D], F32R, kind="ExternalInput")
    ones8d = nc.dram_tensor("ones8", [P, 2, 16], FP8, kind="ExternalInput")
    outd = nc.dram_tensor("outT", [P, DC, R], BF16, kind="ExternalOutput")

    with tile.TileContext(nc) as tc, ExitStack() as top:
        const = top.enter_context(tc.tile_pool(name="const", bufs=1))
        pers = top.enter_context(tc.tile_pool(name="pers", bufs=1))

        # ---- constants resident for the whole kernel ----
        wa_t = const.tile([P, DC, D], FP8)
        ones8_t = const.tile([P, 2, 16], FP8)
        eps1 = const.tile([1, 1], F32)
        nc.vector.memset(eps1[:], LN_EPS)
        onesA_t = const.tile([1, P], F32R)
        cqn_t = const.tile([1, D], F32R)
        dq_t = const.tile([P, DC], F32)
        mask_t = const.tile([P, HCOL], F32)
        wov_t = const.tile([P, DC, D], FP8)
        w1_t = const.tile([P, DC, D2], FP8)
        w2_t = const.tile([P, D2C, D], FP8)
        g2r_t = const.tile([1, D], F32R)
        b1c_t = const.tile([P, D2C], F32)
        b2c_t = const.tile([P, DC], F32)
        be2_t = const.tile([P, DC], F32)

        nc.sync.dma_start(wa_t[:], wad[:])
        nc.sync.dma_start(ones8_t[:], ones8d[:])
        nc.sync.dma_start(onesA_t[:], onesAd[:])
        nc.sync.dma_start(cqn_t[:], cqnd[:])
        nc.sync.dma_start(dq_t[:], dqd[:])

        def load_late_consts():
            nc.sync.dma_start(mask_t[:], maskd[:])
            nc.sync.dma_start(wov_t[:], wovd[:])
            nc.sync.dma_start(w1_t[:], w1d[:])
            nc.sync.dma_start(w2_t[:], w2d[:])
            nc.sync.dma_start(g2r_t[:], g2rd[:])
            nc.sync.dma_start(b1c_t[:], b1cd[:])
            nc.sync.dma_start(b2c_t[:], b2cd[:])
            nc.sync.dma_start(be2_t[:], be2d[:])

        # ---- per-core resident activations ----
        xq8_t = pers.tile([P, DC, R], FP8)
        xb16_t = pers.tile([P, DC, R], BF16)
        q2_t = pers.tile([P, DC, R], FP8)      # scaled by S_Q
        attb_t = pers.tile([P, DC, R], BF16)   # attended, feature-major
        att8_t = pers.tile([P, DC, R], FP8)

        nc.sync.dma_start(xq8_t[:], xq8d[:])
        nc.sync.dma_start(xb16_t[:], xb16d[:])

        def ln_stats(pool, stats_ps, src8, sq8, tag, bufs=2):
            """LN stats of one [128, DC, RB] fp8 chunk -> (rs, sh) f32r rows.

            src8: fp8 source; sq8: same-shape scratch for squares.
            """
            nc.gpsimd.tensor_tensor(sq8[:], src8, src8, OP.mult)
            ps = stats_ps.tile([16, 2, RB], F32, tag="st", bufs=bufs)
            for pr in range(2):
                nc.tensor.matmul(ps[:, 0], ones8_t[:],
                                 src8[:, 2 * pr : 2 * pr + 2],
                                 start=pr == 0, stop=pr == 1, perf_mode=DR)
            for pr in range(2):
                nc.tensor.matmul(ps[:, 1], ones8_t[:],
                                 sq8[:, 2 * pr : 2 * pr + 2],
                                 start=pr == 0, stop=pr == 1, perf_mode=DR)
            st = pool.tile([1, 3, RB], F32, tag=f"st{tag}", bufs=3)
            mu, m2, var = st[:, 0], st[:, 1], st[:, 2]
            nc.vector.tensor_scalar(mu, ps[0:1, 0], 1.0 / D, None, OP.mult)
            nc.gpsimd.tensor_tensor(m2, mu, mu, OP.mult)
            nc.vector.scalar_tensor_tensor(var, ps[0:1, 1], 1.0 / D, m2,
                                           OP.mult, OP.subtract)
            std = st[:, 1]  # reuse m2 slot
            nc.scalar.activation(std, var, AF.Sqrt, bias=eps1[:])
            rsf = st[:, 2]  # reuse var slot
            nc.vector.reciprocal_approx_fast(rsf, std)
            rssh = pool.tile([1, 2, RB], F32R, tag=f"rs{tag}", bufs=3)
            with nc.allow_low_precision(reason="f32r matmul operand"):
                nc.gpsimd.tensor_copy(rssh[:, 0], rsf)
                nc.gpsimd.tensor_tensor(rssh[:, 1], mu, rsf, OP.mult)
            return rssh

        # ================= Stage A: LN1 + Q2 projection =================
        with ExitStack() as ctx:
            apool = ctx.enter_context(tc.tile_pool(name="stage_a", bufs=2))
            stats_ps = ctx.enter_context(
                tc.tile_pool(name="a_stats", bufs=2, space="PSUM"))
            bc_ps = ctx.enter_context(
                tc.tile_pool(name="a_bcast", bufs=2, space="PSUM"))
            mm_ps = ctx.enter_context(
                tc.tile_pool(name="a_mm", bufs=2, space="PSUM"))

            astate = {}

            def a_front(rb):
                rsl = slice(rb * RB, (rb + 1) * RB)
                sq8 = apool.tile([P, DC, RB], FP8, tag="sq", bufs=2)
                astate[rb] = ln_stats(apool, stats_ps, xq8_t[:, :, rsl],
                                      sq8, "a")

            def a_back(rb):
                rsl = slice(rb * RB, (rb + 1) * RB)
                rssh = astate.pop(rb)
                ps_rsb = bc_ps.tile([P, RB], F32, tag="bc", bufs=2)
                nc.tensor.matmul(ps_rsb[:], onesA_t[:], rssh[:, 0],
                                 start=True, stop=True)
                rsb = apool.tile([P, RB], F32, tag="rsb", bufs=2)
                nc.scalar.copy(rsb[:], ps_rsb[:])
                for oc in range(DC):
                    ps_y = mm_ps.tile([P, RB], F32, tag="mm", bufs=2)
                    for pr in range(2):
                        nc.tensor.matmul(
                            ps_y[:], wa_t[:, 2 * pr : 2 * pr + 2,
                                          oc * P : (oc + 1) * P],
                            xq8_t[:, 2 * pr : 2 * pr + 2, rsl],
                            start=pr == 0, stop=pr == 1, perf_mode=DR)
                    ps_csh = bc_ps.tile([P, RB], F32, tag="bc", bufs=2)
                    nc.tensor.matmul(ps_csh[:],
                                     cqn_t[:, oc * P : (oc + 1) * P],
                                     rssh[:, 1], start=True, stop=True)
                    t = apool.tile([P, RB], F32, tag="t", bufs=2)
                    nc.vector.tensor_tensor(t[:], ps_y[:], rsb[:], OP.mult)
                    with nc.allow_low_precision(reason="fp8 activation"):
                        nc.vector.scalar_tensor_tensor(
                            q2_t[:, oc, rsl], t[:], dq_t[:, oc : oc + 1],
                            ps_csh[:], OP.add, OP.add)

            a_front(0)
            a_front(1)
            load_late_consts()
            for rb in range(N_RB):
                if rb + 2 < N_RB:
                    a_front(rb + 2)
                a_back(rb)

        # ================= Stage B/C: attention =================
        with ExitStack() as ctx:
            hpool = ctx.enter_context(tc.tile_pool(name="attn_sb", bufs=1))
            sc_ps = ctx.enter_context(
                tc.tile_pool(name="scores", bufs=2, space="PSUM"))
            ah_ps = ctx.enter_context(
                tc.tile_pool(name="attend", bufs=2, space="PSUM"))

            for blk in range(N_BLK):
                c0 = blk * HCOL
                r0 = blk * P

                hf = hpool.tile([P, DC, HCOL], FP8, tag="hf", bufs=3)
                nc.sync.dma_start(hf[:], hfmd[:, :, c0 : c0 + HCOL])
                hrt = hpool.tile([P, D2C, D], FP8, tag="hr", bufs=3)
                nc.sync.dma_start(hrt[:], hrmd[:, blk * D2C : (blk + 1) * D2C, :])

                ps_sc = sc_ps.tile([P, HCOL], F32, tag="sc", bufs=2)
                for nb in range(2):
                    for pr in range(2):
                        nc.tensor.matmul(
                            ps_sc[:, nb * RB : (nb + 1) * RB],
                            q2_t[:, 2 * pr : 2 * pr + 2, r0 : r0 + P],
                            hf[:, 2 * pr : 2 * pr + 2, nb * RB : (nb + 1) * RB],
                            start=pr == 0, stop=pr == 1, perf_mode=DR)
                nc.vector.tensor_tensor(ps_sc[:], ps_sc[:], mask_t[:], OP.add)

                am = hpool.tile([P, HCOL], BF16, tag="am", bufs=3)
                den = hpool.tile([P, 2], F32, tag="den", bufs=4)
                with nc.allow_low_precision(reason="bf16 attn probs"):
                    nc.scalar.activation(am[:], ps_sc[:], AF.Exp,
                                         scale=1.0 / S_Q,
                                         accum_out=den[:, 0:1])
                nc.vector.reciprocal_approx_fast(den[:, 1:2], den[:, 0:1])

                amT = hpool.tile([P, D2C, P], BF16, tag="amT", bufs=3)
                nc.scalar.dma_start(amT[:], am[:], transpose=True)
                amT8 = hpool.tile([P, D2C, P], FP8, tag="amT8", bufs=3)
                with nc.allow_low_precision(reason="fp8 attn probs"):
                    nc.vector.tensor_copy(amT8[:], amT[:])

                ps_ah = ah_ps.tile([P, D], F32, tag="ah", bufs=2)
                for pr in range(4):
                    nc.tensor.matmul(ps_ah[:], amT8[:, 2 * pr : 2 * pr + 2],
                                     hrt[:, 2 * pr : 2 * pr + 2],
                                     start=pr == 0, stop=pr == 3, perf_mode=DR)
                atb = hpool.tile([P, D], BF16, tag="atb", bufs=3)
                with nc.allow_low_precision(reason="bf16 attended"):
                    nc.vector.tensor_scalar_mul(atb[:], ps_ah[:], den[:, 1:2])
                nc.scalar.dma_start(attb_t[:, :, r0 : r0 + P], atb[:],
                                    transpose=True)
                with nc.allow_low_precision(reason="fp8 attended"):
                    nc.vector.tensor_copy(att8_t[:, :, r0 : r0 + P],
                                          attb_t[:, :, r0 : r0 + P])

        # ================= Stage D/E: Wov + LN2 + FFN =================
        with ExitStack() as ctx:
            epool = ctx.enter_context(tc.tile_pool(name="stage_e", bufs=2))
            stats_ps = ctx.enter_context(
                tc.tile_pool(name="e_stats", bufs=1, space="PSUM"))
            bc_ps = ctx.enter_context(
                tc.tile_pool(name="e_bcast", bufs=1, space="PSUM"))
            mm_ps = ctx.enter_context(
                tc.tile_pool(name="e_mm", bufs=3, space="PSUM"))

            estate = {}

            def e_front(rb):
                rsl = slice(rb * RB, (rb + 1) * RB)
                x2 = epool.tile([P, DC, RB], F32, tag="x2", bufs=3)
                for oc in range(DC):
                    ps_w = mm_ps.tile([P, RB], F32, tag="mm", bufs=3)
                    for pr in range(2):
                        nc.tensor.matmul(
                            ps_w[:], wov_t[:, 2 * pr : 2 * pr + 2,
                                           oc * P : (oc + 1) * P],
                            att8_t[:, 2 * pr : 2 * pr + 2, rsl],
                            start=pr == 0, stop=pr == 1, perf_mode=DR)
                    nc.vector.scalar_tensor_tensor(
                        x2[:, oc], ps_w[:], 1.0 / S_O, xb16_t[:, oc, rsl],
                        OP.mult, OP.add)
                x28 = epool.tile([P, DC, RB], FP8, tag="x28", bufs=2)
                with nc.allow_low_precision(reason="fp8 stats input"):
                    nc.scalar.copy(x28[:], x2[:])
                sq8 = epool.tile([P, DC, RB], FP8, tag="sq", bufs=2)
                estate[rb] = (x2,) + (ln_stats(epool, stats_ps, x28[:],
                                               sq8, "e", bufs=1),)

            def e_back(rb):
                rsl = slice(rb * RB, (rb + 1) * RB)
                x2, rssh = estate.pop(rb)
                h28 = epool.tile([P, DC, RB], FP8, tag="h2", bufs=2)
                for dc in range(DC):
                    ps_ab = bc_ps.tile([P, 2, RB], F32, tag="bc", bufs=1)
                    nc.tensor.matmul(ps_ab[:, 0],
                                     g2r_t[:, dc * P : (dc + 1) * P],
                                     rssh[:, 0], start=True, stop=True)
                    nc.tensor.matmul(ps_ab[:, 1],
                                     g2r_t[:, dc * P : (dc + 1) * P],
                                     rssh[:, 1], start=True, stop=True)
                    with nc.allow_low_precision(reason="fp8 LN2 out"):
                        nc.vector.tensor_tensor(h28[:, dc], x2[:, dc],
                                                ps_ab[:, 0], OP.mult)
                        nc.vector.scalar_tensor_tensor(
                            h28[:, dc], h28[:, dc], be2_t[:, dc : dc + 1],
                            ps_ab[:, 1], OP.add, OP.subtract)

                a18 = epool.tile([P, D2C, RB], FP8, tag="a1", bufs=2)
                for oc in range(D2C):
                    ps_f = mm_ps.tile([P, RB], F32, tag="mm", bufs=3)
                    for pr in range(2):
                        nc.tensor.matmul(
                            ps_f[:], w1_t[:, 2 * pr : 2 * pr + 2,
                                          oc * P : (oc + 1) * P],
                            h28[:, 2 * pr : 2 * pr + 2],
                            start=pr == 0, stop=pr == 1, perf_mode=DR)
                    with nc.allow_low_precision(reason="fp8 gelu"):
                        nc.scalar.activation(a18[:, oc], ps_f[:], AF.Gelu,
                                             bias=b1c_t[:, oc : oc + 1],
                                             scale=1.0 / S_1)

                ot = epool.tile([P, DC, RB], BF16, tag="ot", bufs=2)
                tf = epool.tile([P, RB], F32, tag="tf", bufs=3)
                for oc in range(DC):
                    ps_f = mm_ps.tile([P, RB], F32, tag="mm", bufs=3)
                    for pr in range(4):
                        nc.tensor.matmul(
                            ps_f[:], w2_t[:, 2 * pr : 2 * pr + 2,
                                          oc * P : (oc + 1) * P],
                            a18[:, 2 * pr : 2 * pr + 2],
                            start=pr == 0, stop=pr == 3, perf_mode=DR)
                    nc.scalar.activation(tf[:], ps_f[:], AF.Identity,
                                         bias=b2c_t[:, oc : oc + 1],
                                         scale=1.0 / S_2)
                    with nc.allow_low_precision(reason="bf16 output"):
                        nc.vector.tensor_tensor(ot[:, oc], tf[:], x2[:, oc],
                                                OP.add)
                nc.sync.dma_start(outd[:, :, rsl], ot[:])

            e_front(0)
            e_front(1)
            for rb in range(N_RB):
                if rb + 2 < N_RB:
                    e_front(rb + 2)
                e_back(rb)

    nc.compile()
    return nc


def _tile_fm(a, dt):
    """[Dred, N] feature-major -> [128, Dred//128, N] device tiling."""
    dred, n = a.shape
    return np.ascontiguousarray(
        a.reshape(dred // P, P, n).swapaxes(0, 1)).astype(dt)


def _vec_pc(v):
    """[n*128] vector -> [128, n] (partition, chunk) f32."""
    return np.ascontiguousarray(np.asarray(v, np.float64).reshape(-1, P).T
                                ).astype(np.float32)


def _make_weight_map(inputs):
    f64 = {k: np.asarray(v, np.float64) for k, v in inputs.items()}
    isd = 1.0 / math.sqrt(D)

    WQK = (f64["Wq"].T @ f64["Wk"]) * isd          # [f, e]
    WA = f64["g1"][:, None] * WQK
    cq = WA.sum(axis=0)                            # [e]
    dq = f64["be1"] @ WQK + f64["bq"] @ f64["Wk"] * isd
    WOV = f64["Wo"] @ f64["Wv"]                    # [d, f]
    bo_p = f64["bo"] + f64["Wo"] @ f64["bv"]

    wa8 = _tile_fm(WA * S_A, ml_dtypes.float8_e4m3)
    wov8 = _tile_fm(WOV.T * S_O, ml_dtypes.float8_e4m3)
    w18 = _tile_fm(f64["W1"].T * S_1, ml_dtypes.float8_e4m3)
    w28 = _tile_fm(f64["W2"].T * S_2, ml_dtypes.float8_e4m3)

    h = np.arange(H)
    logdecay = np.log(DECAY_RATE ** (H - 1 - h) + 1e-10)
    maskS = np.full((P, HCOL), NEG_BIG * S_Q, np.float64)
    for p_ in range(P):
        t = p_ // B
        maskS[p_, t * H : (t + 1) * H] = S_Q * logdecay
    ones8 = np.ones((P, 2, 16), ml_dtypes.float8_e4m3)

    return dict(
        wa8=wa8, wov8=wov8, w18=w18, w28=w28,
        maskS=maskS.astype(np.float32),
        dq=_vec_pc(dq * S_Q),
        b1c=_vec_pc(f64["b1"]),
        b2c=_vec_pc(f64["b2"]),
        be2c=_vec_pc(f64["be2"]),
        onesA=np.full((1, P), S_Q / S_A, np.float32),
        cqn=(-cq * S_Q)[None, :].astype(np.float32),
        g2r=np.ascontiguousarray(f64["g2"][None, :]).astype(np.float32),
        ones8=ones8,
        _bo_p=bo_p,  # consumed by core_input_map, not a dram tensor
    )


def core_input_map(inputs, wmap, c):
    """Per-core input dict (core c owns positions [c*T_LOC, (c+1)*T_LOC))."""
    x = np.asarray(inputs["x"], np.float32)
    history = np.asarray(inputs["history"], np.float32)
    ts = slice(c * T_LOC, (c + 1) * T_LOC)
    xr = x[:, ts, :].transpose(1, 0, 2).reshape(R, D)      # r = t*B + b
    hr = history[ts].reshape(HR, D)
    m = {k: v for k, v in wmap.items() if not k.startswith("_")}
    m["xq8"] = _tile_fm(np.ascontiguousarray(xr.T), ml_dtypes.float8_e4m3)
    m["xb16"] = _tile_fm(np.ascontiguousarray((xr + wmap["_bo_p"]).T),
                         ml_dtypes.bfloat16)
    m["hfm8"] = _tile_fm(np.ascontiguousarray(hr.T), ml_dtypes.float8_e4m3)
    m["hrm8"] = np.ascontiguousarray(
        hr.reshape(HR // P, P, D).swapaxes(0, 1)).astype(ml_dtypes.float8_e4m3)
    return m


def unpack_out(ot):
    """[128, DC, R] bf16 device tile -> [B, T_LOC, D] f32."""
    full = np.asarray(ot, np.float32).swapaxes(0, 1).reshape(D, R).T
    return full.reshape(T_LOC, B, D).transpose(1, 0, 2)


def kernel(**inputs):
    if "nc" not in _cached:
        _cached["nc"] = _build_program()
    nc = _cached["nc"]

    wmap = _make_weight_map(inputs)
    in_maps = [core_input_map(inputs, wmap, c) for c in range(N_CORES)]

    res = run_bass_kernel_spmd(nc, in_maps, core_ids=list(range(N_CORES)))
    _last_result[0] = res

    out = np.empty((B, T, D), dtype=np.float32)
    for c in range(N_CORES):
        ts = slice(c * T_LOC, (c + 1) * T_LOC)
        out[:, ts, :] = unpack_out(res.results[c]["outT"])
    return out
